# revision 1
# baseline (speedup 1.0000x reference)
"""Trainium2 Bass kernel for nn_DAE_44779329028610 (embedding autoencoder).

  y = sigmoid(sigmoid(x @ w + b) @ w.T)
  x [4096, 81616] f32, w [81616, 32] f32, b [32] f32 -> y [4096, 81616] f32

Strategy: data-parallel shard of the batch dim across 8 NeuronCores
(512 rows/core); w and b replicated. Per core, a two-pass Tile kernel:

Pass 1 (encoder): stream x in [128, 2048]-tiles (one 4 MiB DMA covers all
four batch tiles of a vocab super-chunk); PE-transpose each 128x128 block
(f32 transpose-mode matmuls, identity stationary) into PSUM; evict
PSUM->SBUF with an f32->f32r rounding copy (walrus requires fp32r matmul
operands to come from an explicit rounding producer); accumulate
hT[32, 512] in one PSUM bank over all 638 vocab chunks with the w-chunk
[128v, 32] (also rounded to f32r) as the stationary. fp32r matmuls run at
1 cycle/row for N>=256 vs 4 for plain f32, at ~1e-4 relative precision.
Each w-chunk is additionally PE-transposed, staged in SBUF, and DMA'd
into a persistent wT store laid out in 4 partition groups (vocab quarter
g lives on partitions 32g..32g+32) for pass 2.

Pass 2 (decoder): hT (sigmoid+bias applied on ACT while reading PSUM,
rounded to f32r, replicated to all 4 partition groups by SBUF->SBUF DMA)
is the stationary [32, 128]; wT chunks [32, 512] are the moving operand;
K=32 matmuls run in PE row group g (tile_position=(32g, 0)); ACT applies
sigmoid PSUM->SBUF; y leaves in [128, 4096]-tiles (2 MiB DMAs).

The workload is HBM-bound (read 1.34 GB of x + write 1.34 GB of y); the
kernel keeps every compute engine well under the per-core DMA time
(~0.5 ms in + ~0.5 ms out at ~360 GB/s), reads w only once, and never
spills intermediates to DRAM.
"""

import sys

if "/opt/trn_rl_repo" not in sys.path:
    sys.path.insert(0, "/opt/trn_rl_repo")

from contextlib import ExitStack

import numpy as np

from concourse import bacc, masks, mybir, tile
from concourse.bass_utils import run_bass_kernel_spmd

# The neuronx_cc hook recompiles the NEFF from scratch in every process
# (~5 min of walrus for this kernel). Cache the compiled NEFF on disk,
# keyed by the BIR hash, so repeat runs are instant.
import hashlib
import os
import shutil

import concourse.bass2jax as _bass2jax

_NEFF_CACHE_DIR = "/tmp/bass_neff_cache"
_orig_compile_bir_kernel = _bass2jax.compile_bir_kernel


def _cached_compile_bir_kernel(bir_json, tmpdir, neff_name="file.neff"):
    os.makedirs(_NEFF_CACHE_DIR, exist_ok=True)
    key = hashlib.sha256(bir_json).hexdigest()[:32]
    cpath = os.path.join(_NEFF_CACHE_DIR, f"{key}.neff")
    out = os.path.join(tmpdir, neff_name)
    if os.path.exists(cpath):
        shutil.copyfile(cpath, out)
        return out
    out = _orig_compile_bir_kernel(bir_json, tmpdir, neff_name)
    try:
        shutil.copyfile(out, cpath)
    except OSError:
        pass
    return out


_bass2jax.compile_bir_kernel = _cached_compile_bir_kernel

F32 = mybir.dt.float32
F32R = mybir.dt.float32r

B_FULL = 4096
V = 81616
D = 32
N_CORES = 8
B_CORE = B_FULL // N_CORES


def _ceil_div(a, b):
    return -(-a // b)


def build_dae(B_core, V, S=2048, y_tile_chunks=8, x_bufs=2, xt_bufs=3, y_bufs=4, repeat=1):
    """Build + compile the per-core Bass program. S = vocab super-chunk."""
    assert B_core % 128 == 0
    nbt = B_core // 128  # batch tiles per core
    NB = nbt * 128
    assert NB <= 512
    assert S % 128 == 0

    # vocab quarters (wT partition groups), multiples of 512
    qb = max(512, _ceil_div(_ceil_div(V, 4), 512) * 512)
    quarters = []  # (start, size)
    for g in range(4):
        s = min(g * qb, V)
        e = min((g + 1) * qb, V)
        quarters.append((s, e - s))

    nc = bacc.Bacc("TRN2", target_bir_lowering=False, debug=False)

    x_d = nc.dram_tensor("x", [B_core, V], F32, kind="ExternalInput")
    w_d = nc.dram_tensor("w", [V, D], F32, kind="ExternalInput")
    b_d = nc.dram_tensor("b", [D], F32, kind="ExternalInput")
    y_d = nc.dram_tensor("y", [B_core, V], F32, kind="ExternalOutput")

    n_chunks_total = _ceil_div(V, 128)  # encoder 128-chunks

    with tile.TileContext(nc) as tc, ExitStack() as ctx:
        const_pool = ctx.enter_context(tc.tile_pool(name="const", bufs=1))
        ident = const_pool.tile([128, 128], F32)
        masks.make_identity(nc, ident[:])
        b_sb = const_pool.tile([D, 1], F32)
        nc.sync.dma_start(b_sb[:, 0:1], b_d[:].unsqueeze(1))

        # persistent stores: wT quarters + replicated hT, both f32r
        wt_pool = ctx.enter_context(tc.tile_pool(name="wt", bufs=1))
        wT = wt_pool.tile([128, qb], F32R)
        hT_rep = wt_pool.tile([128, NB], F32R)

        def _passes():
            # ---------------- pass 1: encoder ----------------
            with ExitStack() as p1:
                xpool = p1.enter_context(tc.tile_pool(name="x", bufs=x_bufs))
                wpool = p1.enter_context(tc.tile_pool(name="w", bufs=2))
                xtpool = p1.enter_context(tc.tile_pool(name="xt", bufs=xt_bufs))
                ps_x = p1.enter_context(tc.tile_pool(name="psx", bufs=3, space="PSUM"))
                ps_w = p1.enter_context(tc.tile_pool(name="psw", bufs=2, space="PSUM"))
                ps_h = p1.enter_context(tc.tile_pool(name="psh", bufs=1, space="PSUM"))

                hT_ps = ps_h.tile([D, NB], F32)

                chunk_i = 0  # global 128-chunk index
                for v0 in range(0, V, S):
                    sl = min(S, V - v0)  # super-chunk len
                    n_sub = _ceil_div(sl, 128)
                    # x: one DMA for all batch tiles of this super-chunk
                    x_t = xpool.tile([128, nbt, sl], F32)
                    nc.sync.dma_start(
                        x_t[:], x_d[:, v0 : v0 + sl].rearrange("(t p) v -> p t v", p=128)
                    )
                    # w rows v0:v0+sl scattered as [128, n_sub, D]
                    w_t = wpool.tile([128, n_sub, D], F32)
                    nfull = sl // 128
                    rem = sl - nfull * 128
                    if nfull:
                        nc.sync.dma_start(
                            w_t[:, 0:nfull, :],
                            w_d[v0 : v0 + nfull * 128, :].rearrange(
                                "(c p) d -> p c d", p=128
                            ),
                        )
                    if rem:
                        nc.sync.dma_start(
                            w_t[0:rem, nfull, :],
                            w_d[v0 + nfull * 128 : v0 + sl, :],
                        )
                    # rounded copy of w for f32r matmuls (written regions only)
                    w_r = wpool.tile([128, n_sub, D], F32R, tag="w_r")
                    if nfull:
                        nc.scalar.copy(w_r[:, 0:nfull, :], w_t[:, 0:nfull, :])
                    if rem:
                        nc.scalar.copy(w_r[0:rem, nfull, :], w_t[0:rem, nfull, :])
                    # staging tile for this super-chunk's slice of wT (f32r,
                    # partitions 0..32; DMA'd to the right partition group below)
                    wt_stage = wpool.tile([D, S], F32R, tag="wt_stage")
                    for c in range(n_sub):
                        vlen = min(128, sl - c * 128)
                        # transpose nbt x-blocks into one psum tile (plain f32)
                        xT_ps = ps_x.tile([128, NB], F32)
                        for t in range(nbt):
                            nc.tensor.matmul(
                                xT_ps[0:vlen, t * 128 : (t + 1) * 128],
                                x_t[:, t, c * 128 : c * 128 + vlen],
                                ident[:, 0:128],
                                is_transpose=True,
                            )
                        # evict + round to f32r
                        xT_sb = xtpool.tile([128, NB], F32R)
                        nc.vector.tensor_copy(xT_sb[0:vlen, :], xT_ps[0:vlen, :])
                        # accumulate hT += w_chunk.T @ xT_chunk   (f32r, N=NB)
                        nc.tensor.matmul(
                            hT_ps[:, :],
                            w_r[0:vlen, c, :],
                            xT_sb[0:vlen, :],
                            start=(chunk_i == 0),
                            stop=(chunk_i == n_chunks_total - 1),
                        )
                        # transpose w-chunk for the decoder (psum partition 0
                        # only; walrus forbids transpose outputs elsewhere)
                        wT_ps = ps_w.tile([D, 128], F32)
                        nc.tensor.matmul(
                            wT_ps[0:D, 0:vlen],
                            w_t[0:vlen, c, :],
                            ident[0:vlen, 0:vlen],
                            is_transpose=True,
                        )
                        nc.scalar.copy(
                            wt_stage[0:D, c * 128 : c * 128 + vlen],
                            wT_ps[0:D, 0:vlen],
                        )
                        chunk_i += 1
                    # move staged wT slice to its partition group(s); a
                    # super-chunk may straddle a quarter boundary
                    seg = v0
                    while seg < v0 + sl:
                        g = seg // qb
                        seg_end = min((g + 1) * qb, v0 + sl)
                        nc.sync.dma_start(
                            wT[32 * g : 32 * g + D, seg - g * qb : seg_end - g * qb],
                            wt_stage[0:D, seg - v0 : seg_end - v0],
                        )
                        seg = seg_end

                # hT = sigmoid(hT_pre + b); round to f32r; replicate to groups
                hT_f32 = const_pool.tile([D, NB], F32)
                nc.scalar.activation(
                    hT_f32[:, :],
                    hT_ps[:, :],
                    mybir.ActivationFunctionType.Sigmoid,
                    bias=b_sb[:, 0:1],
                )
                nc.any.tensor_copy(hT_rep[0:D, :], hT_f32[:, :])
                for g in range(1, 4):
                    nc.sync.dma_start(hT_rep[32 * g : 32 * g + D, :], hT_rep[0:D, :])

            # ---------------- pass 2: decoder ----------------
            with ExitStack() as p2:
                ypool = p2.enter_context(tc.tile_pool(name="y", bufs=y_bufs))
                ps_y = p2.enter_context(tc.tile_pool(name="psy", bufs=6, space="PSUM"))
                YS = 512 * y_tile_chunks  # y sbuf tile free size
                for t in range(nbt):
                    for g in range(4):
                        q0, qlen = quarters[g]
                        if qlen == 0:
                            continue
                        for yo in range(0, qlen, YS):
                            ylen = min(YS, qlen - yo)
                            y_sb = ypool.tile([128, YS], F32)
                            for co in range(0, ylen, 512):
                                nlen = min(512, ylen - co)
                                y_ps = ps_y.tile([128, 512], F32)
                                nc.tensor.matmul(
                                    y_ps[:, 0:nlen],
                                    hT_rep[32 * g : 32 * g + D, t * 128 : (t + 1) * 128],
                                    wT[32 * g : 32 * g + D, yo + co : yo + co + nlen],
                                    tile_position=(32 * g, 0),
                                )
                                nc.scalar.activation(
                                    y_sb[:, co : co + nlen],
                                    y_ps[:, 0:nlen],
                                    mybir.ActivationFunctionType.Sigmoid,
                                )
                            nc.sync.dma_start(
                                y_d[t * 128 : (t + 1) * 128, q0 + yo : q0 + yo + ylen],
                                y_sb[:, 0:ylen],
                            )

        if repeat == 1:
            _passes()
        else:
            # timing aid: run the whole two-pass kernel `repeat` times on
            # device inside one NEFF (For_i back-edge ~2us per iteration)
            with tc.For_i(0, repeat, 1):
                _passes()

    nc.compile()
    return nc


_NC_CACHE = None


def _get_nc():
    global _NC_CACHE
    if _NC_CACHE is None:
        _NC_CACHE = build_dae(B_CORE, V)
    return _NC_CACHE


def _in_maps(x, w, b):
    x = np.ascontiguousarray(x, dtype=np.float32)
    w = np.ascontiguousarray(w, dtype=np.float32)
    b = np.ascontiguousarray(b, dtype=np.float32)
    return [
        {"x": x[i * B_CORE : (i + 1) * B_CORE], "w": w, "b": b}
        for i in range(N_CORES)
    ]


def kernel(x, w, b):
    assert x.shape == (B_FULL, V) and w.shape == (V, D) and b.shape == (D,)
    nc = _get_nc()
    in_maps = _in_maps(x, w, b)
    last = None
    # the first execution of a freshly compiled NEFF on this axon terminal
    # occasionally reports NRT_EXEC_UNIT_UNRECOVERABLE; a retry succeeds
    for _ in range(3):
        try:
            res = run_bass_kernel_spmd(nc, in_maps, core_ids=list(range(N_CORES)))
            break
        except Exception as e:  # noqa: BLE001
            last = e
    else:
        raise last
    return np.concatenate([res.results[i]["y"] for i in range(N_CORES)], axis=0)

